# revision 1
# baseline (speedup 1.0000x reference)
"""Coupled-attention module as a distributed Bass/Tile kernel on 8 TRN2 cores.

Math notes (exact algebra, not approximations):
- The differential-attention scores are constant along the softmax axis, so
  softmax yields exactly uniform 1/S weights: diff_vector collapses to the
  per-batch mean of (y @ dv_w + dv_b), broadcast over sequence. dq/dk are dead.
- Sharding: rows of the flattened (B*S, H) activations, 256 per core; cores
  0-3 own batch 0, 4-7 batch 1. Each core redundantly computes full-batch K/V
  (cheaper than any reshard collective at this scale).
- All activations live channel-major [C, rows] on chip, so weights feed the
  PE as natural [K, M] lhsT tiles, and the two sequence-axis softmaxes in the
  gating network reduce along the free dim. Their denominators are summed
  across the 4-core batch group with tiny AllReduces.
- Compute in bf16 with fp32 accumulation (all GEMMs), exp/tanh/sigmoid on ACT.
- The AllReduce-independent halves of the v_gamma and van_out GEMMs are
  pre-accumulated into SBUF while the collectives are in flight, keeping the
  PE busy (and its HAM clock warm) through the bubbles.
"""

import numpy as np
import ml_dtypes

import concourse.bass as bass
import concourse.mybir as mybir
import concourse.tile as tile
from concourse import bacc
from concourse.bass_utils import run_bass_kernel_spmd

B, S, H = 2, 1024, 768
NH, DH = 12, 64
P = 128
RV = 256            # rows per core
KC = H // P         # 6 channel chunks
JC = S // P         # 8 sequence chunks
GROUPS = [[0, 1, 2, 3], [4, 5, 6, 7]]
SCALE = 1.0 / 8.0   # 1/sqrt(DH)

bf16 = mybir.dt.bfloat16
f32 = mybir.dt.float32
AF = mybir.ActivationFunctionType
ALU = mybir.AluOpType
nbf16 = ml_dtypes.bfloat16

W768 = ["vq_w", "vk_w", "vv_w", "dv_w", "WD_w", "van_fc_w", "WV_w", "diff_fc_w",
        "diff_fus_w", "van_fus_w", "nf_w", "final_w"]
W1536 = ["d_theta_w", "v_gamma_w", "diff_out_w", "van_out_w"]
BIAS = ["vq_b", "vk_b", "dv_b", "van_fc_b", "d_theta_b", "diff_fc_b",
        "v_gamma_b", "diff_out_b", "van_out_b", "diff_fus_b", "van_fus_b",
        "nf_b", "final_b"]


def build(has_vvb: bool):
    nc = bacc.Bacc(None, target_bir_lowering=False, debug=False, num_devices=8)

    xT_d = nc.dram_tensor("xT", [H, RV], bf16, kind="ExternalInput")
    yT_d = nc.dram_tensor("yT", [H, S], bf16, kind="ExternalInput")
    wd = {}
    for w in W768:
        wd[w] = nc.dram_tensor(w, [H, H], bf16, kind="ExternalInput")
    for w in W1536:
        wd[w] = nc.dram_tensor(w, [2 * H, H], bf16, kind="ExternalInput")
    wd["gate_w"] = nc.dram_tensor("gate_w", [2 * H, 1], bf16, kind="ExternalInput")
    wd["nf_out_w"] = nc.dram_tensor("nf_out_w", [2 * H, 1], bf16, kind="ExternalInput")
    bd = {}
    for b in BIAS:
        bd[b] = nc.dram_tensor(b, [H], f32, kind="ExternalInput")
    if has_vvb:
        bd["vv_b"] = nc.dram_tensor("vv_b", [H], f32, kind="ExternalInput")
    out_d = nc.dram_tensor("outT", [H, RV], f32, kind="ExternalOutput")

    with tile.TileContext(nc, num_cores=8) as tc:
        with (
            tc.tile_pool(name="wpool", bufs=5) as wp,
            tc.tile_pool(name="wsmall", bufs=2) as wsp,
            tc.tile_pool(name="acts", bufs=1) as ap,
            tc.tile_pool(name="loop", bufs=2) as lp,
            tc.tile_pool(name="psum", bufs=8, space="PSUM") as pp,
            tc.tile_pool(name="dram", bufs=4, space="DRAM") as dp,
        ):
            def wtile(name, half=None):
                t = wp.tile([P, KC, H], bf16, name=f"w_{name}_{half}", tag="w")
                src = wd[name]
                if half is not None:
                    src = src[half * H:(half + 1) * H, :]
                src = src.rearrange("(kc p) n -> kc p n", p=P)
                for kc in range(KC):
                    nc.sync.dma_start(t[:, kc, :], src[kc])
                return t

            def btile(name):
                t = ap.tile([P, KC], f32, name=f"b_{name}")
                nc.sync.dma_start(t[:], bd[name].rearrange("(c p) -> p c", p=P))
                return t

            # ---------------- Q projection first: minimal-dependency PE work
            b_vq = btile("vq_b")
            xT = ap.tile([P, KC, RV], bf16, name="xT")
            for kc in range(KC):
                nc.sync.dma_start(xT[:, kc, :], xT_d.rearrange(
                    "(kc p) n -> kc p n", p=P)[kc])
            w_vq = wtile("vq_w")
            qT = ap.tile([P, KC, RV], bf16, name="qT")
            for mc in range(KC):
                ps = pp.tile([P, RV], f32, name=f"qps{mc}", tag="sps", bufs=3)
                for kc in range(KC):
                    nc.tensor.matmul(ps[:], w_vq[:, kc, mc * P:(mc + 1) * P],
                                     xT[:, kc, :],
                                     start=(kc == 0), stop=(kc == KC - 1))
                nc.scalar.activation(qT[:, mc, :], ps[:], AF.Identity,
                                     bias=b_vq[:, mc:mc + 1])

            b_vk = btile("vk_b")
            b_dv = btile("dv_b")
            yT = ap.tile([P, KC, S], bf16, name="yT")
            for kc in range(KC):
                nc.sync.dma_start(yT[:, kc, :], yT_d.rearrange(
                    "(kc p) n -> kc p n", p=P)[kc])

            ones64 = ap.tile([1, 64], f32, name="ones64")
            nc.vector.memset(ones64[:], 1.0)
            ones128 = ap.tile([1, P], f32, name="ones128")
            nc.vector.memset(ones128[:], 1.0)

            # ---------------- K projection (full batch, channel-major) ------
            w_vk = wtile("vk_w")
            kT = ap.tile([P, KC, S], bf16, name="kT")
            for mc in range(KC):
                for nh in range(2):
                    ps = pp.tile([P, 512], f32, name=f"kps{mc}_{nh}", tag="big", bufs=3)
                    for kc in range(KC):
                        nc.tensor.matmul(
                            ps[:], w_vk[:, kc, mc * P:(mc + 1) * P],
                            yT[:, kc, nh * 512:(nh + 1) * 512],
                            start=(kc == 0), stop=(kc == KC - 1))
                    nc.scalar.activation(kT[:, mc, nh * 512:(nh + 1) * 512], ps[:],
                                         AF.Identity, bias=b_vk[:, mc:mc + 1])

            # ---------------- V projection (row-major + ones col) -----------
            w_vv = wtile("vv_w")
            v_aug = ap.tile([P, JC, NH, DH + 1], bf16, name="v_aug")
            nc.vector.memset(v_aug[:, :, :, DH:DH + 1], 1.0)
            for jc in range(JC):
                for cg in range(2):
                    ps = pp.tile([P, 384], f32, name=f"vps{jc}_{cg}", tag="big", bufs=3)
                    for kc in range(KC):
                        nc.tensor.matmul(
                            ps[:], yT[:, kc, jc * P:(jc + 1) * P],
                            w_vv[:, kc, cg * 384:(cg + 1) * 384],
                            start=(kc == 0), stop=(kc == KC - 1))
                    nc.vector.tensor_copy(
                        v_aug[:, jc, cg * 6:(cg + 1) * 6, 0:DH],
                        ps[:].rearrange("p (h d) -> p h d", d=DH))

            # ---------------- diff-branch constants (per batch) -------------
            # m = mean_s(y) @ dv_w + dv_b ; theta1 = tanh(m @ WD_w)
            # bias1 = theta1 @ d_theta_w[:H] + d_theta_b
            # bias2 = m @ diff_out_w[:H] + diff_out_b
            yb = ap.tile([P, KC], f32, name="yb")
            ybt = ap.tile([P, KC], bf16, name="ybt")
            for kc in range(KC):
                nc.vector.tensor_reduce(yb[:, kc:kc + 1], yT[:, kc, :],
                                        axis=mybir.AxisListType.X, op=ALU.add)
            nc.vector.tensor_scalar_mul(ybt[:], yb[:], 1.0 / S)

            def vec_chain(w_t, rhs_t, func, bias_t, out_dt, name):
                out = ap.tile([P, KC], out_dt, name=name)
                for mc in range(KC):
                    ps = pp.tile([P, 1], f32, name=f"{name}ps{mc}", tag="sps", bufs=3)
                    for kc in range(KC):
                        nc.tensor.matmul(ps[:], w_t[:, kc, mc * P:(mc + 1) * P],
                                         rhs_t[:, kc:kc + 1],
                                         start=(kc == 0), stop=(kc == KC - 1))
                    nc.scalar.activation(out[:, mc:mc + 1], ps[:], func,
                                         bias=(bias_t[:, mc:mc + 1]
                                               if bias_t is not None else 0.0))
                return out

            w_dv = wtile("dv_w")
            m32 = vec_chain(w_dv, ybt, AF.Identity, b_dv, f32, "m32")
            mbf = ap.tile([P, KC], bf16, name="mbf")
            nc.vector.tensor_copy(mbf[:], m32[:])
            w_WD = wtile("WD_w")
            th1 = vec_chain(w_WD, mbf, AF.Tanh, None, bf16, "th1")
            w_dth0 = wtile("d_theta_w", half=0)
            b_dth = btile("d_theta_b")
            bias1 = vec_chain(w_dth0, th1, AF.Identity, b_dth, f32, "bias1")
            w_dout0 = wtile("diff_out_w", half=0)
            b_dout = btile("diff_out_b")
            bias2 = vec_chain(w_dout0, mbf, AF.Identity, b_dout, f32, "bias2")

            # ---------------- attention (12 heads, 256 own queries) ---------
            if has_vvb:
                b_vv = btile("vv_b")
            vanT = ap.tile([P, KC, RV], bf16, name="vanT")

            def head_tail(h, pv):
                hc, ho = h // 2, (h % 2) * 64
                invZ = lp.tile([1, RV], f32, name=f"invZ{h}", tag="invZ")
                nc.vector.reciprocal(invZ[:], pv[DH:DH + 1, :])
                bc = pp.tile([64, RV], f32, name=f"bc{h}", tag="sps", bufs=3)
                nc.tensor.matmul(bc[:], ones64[:], invZ[:], start=True, stop=True)
                bcs = lp.tile([64, RV], f32, name=f"bcs{h}", tag="bcs")
                nc.vector.tensor_copy(bcs[:], bc[:])
                nc.vector.tensor_mul(vanT[ho:ho + 64, hc, :], pv[0:DH, :], bcs[:])
                if has_vvb:
                    nc.vector.tensor_scalar_add(vanT[ho:ho + 64, hc, :],
                                                vanT[ho:ho + 64, hc, :],
                                                b_vv[ho:ho + 64, hc:hc + 1])

            for hp in range(NH // 2):
                h0, h1 = 2 * hp, 2 * hp + 1
                hc = hp
                e0 = lp.tile([P, JC, RV], bf16, name=f"expT{h0}", tag="expT", bufs=4)
                e1_ = lp.tile([P, JC, RV], bf16, name=f"expT{h1}", tag="expT", bufs=4)
                pv0 = pp.tile([DH + 1, RV], f32, name=f"pv{h0}", tag="pv", bufs=2)
                pv1 = pp.tile([DH + 1, RV], f32, name=f"pv{h1}", tag="pv", bufs=2)
                for jc in range(JC):
                    for (h, ex) in ((h0, e0), (h1, e1_)):
                        ho = (h % 2) * 64
                        sps = pp.tile([P, RV], f32, name=f"sps{h}_{jc}",
                                      tag="sps", bufs=3)
                        nc.tensor.matmul(sps[:],
                                         kT[ho:ho + 64, hc, jc * P:(jc + 1) * P],
                                         qT[ho:ho + 64, hc, :],
                                         start=True, stop=True)
                        nc.scalar.activation(ex[:, jc, :], sps[:], AF.Exp,
                                             scale=SCALE)
                for jc in range(JC):
                    nc.tensor.matmul(pv0[:], v_aug[:, jc, h0, :], e0[:, jc, :],
                                     start=(jc == 0), stop=(jc == JC - 1))
                    nc.tensor.matmul(pv1[:], v_aug[:, jc, h1, :], e1_[:, jc, :],
                                     start=(jc == 0), stop=(jc == JC - 1))
                head_tail(h0, pv0)
                head_tail(h1, pv1)

            # ---------------- gating network ---------------------------------
            def gemm(pairs, func, bias_t=None, accum_t=None, name="g",
                     out_dt=bf16, pre=None):
                out = ap.tile([P, KC, RV], out_dt, name=name)
                nmm = len(pairs) * KC
                for mc in range(KC):
                    ps = pp.tile([P, RV], f32, name=f"{name}ps{mc}", tag="big", bufs=3)
                    i = 0
                    for wt, at in pairs:
                        for kc in range(KC):
                            nc.tensor.matmul(ps[:],
                                             wt[:, kc, mc * P:(mc + 1) * P],
                                             at[:, kc, :],
                                             start=(i == 0), stop=(i == nmm - 1))
                            i += 1
                    src = ps
                    if pre is not None:
                        tmp = lp.tile([P, RV], f32, name=f"{name}pre{mc}",
                                      tag="pretmp")
                        nc.vector.tensor_add(tmp[:], ps[:], pre[:, mc, :])
                        src = tmp
                    nc.scalar.activation(
                        out[:, mc, :], src[:], func,
                        bias=(bias_t[:, mc:mc + 1] if bias_t is not None else 0.0),
                        accum_out=(accum_t[:, mc:mc + 1]
                                   if accum_t is not None else None))
                return out

            def allreduce6(part, name):
                ci = dp.tile([P, KC], f32, name=f"ci_{name}")
                co = dp.tile([P, KC], f32, name=f"co_{name}")
                nc.sync.dma_start(ci[:], part[:])
                nc.gpsimd.collective_compute(
                    "AllReduce", ALU.add, replica_groups=GROUPS,
                    ins=[ci[:]], outs=[co[:]])
                z = ap.tile([P, KC], f32, name=f"z_{name}")
                nc.sync.dma_start(z[:], co[:])
                return z

            w_vfc = wtile("van_fc_w")
            b_vfc = btile("van_fc_b")
            theta2 = gemm([(w_vfc, vanT)], AF.Tanh, bias_t=b_vfc, name="theta2")

            w_dth1 = wtile("d_theta_w", half=1)
            part1 = ap.tile([P, KC], f32, name="part1")
            e1 = gemm([(w_dth1, theta2)], AF.Exp, bias_t=bias1, accum_t=part1,
                      name="e1")
            z1 = allreduce6(part1, "z1")

            # --- AllReduce-1 bubble fillers (independent of z1) -------------
            w_WV = wtile("WV_w")
            gamma1 = gemm([(w_WV, vanT)], AF.Tanh, name="gamma1")
            w_vg0 = wtile("v_gamma_w", half=0)
            b_vg = btile("v_gamma_b")
            z2a = gemm([(w_vg0, gamma1)], AF.Identity, bias_t=b_vg, name="z2a",
                       out_dt=f32)
            w_vo0 = wtile("van_out_w", half=0)
            b_vo = btile("van_out_b")
            voa = gemm([(w_vo0, vanT)], AF.Identity, bias_t=b_vo, name="voa",
                       out_dt=f32)

            s1 = ap.tile([P, KC], f32, name="s1")
            nc.vector.reciprocal(s1[:], z1[:])
            nc.vector.tensor_mul(s1[:], s1[:], m32[:])
            dth = ap.tile([P, KC, RV], bf16, name="dth")
            for mc in range(KC):
                nc.vector.tensor_scalar_mul(dth[:, mc, :], e1[:, mc, :],
                                            s1[:, mc:mc + 1])

            w_dfc = wtile("diff_fc_w")
            b_dfc = btile("diff_fc_b")
            gamma2 = gemm([(w_dfc, dth)], AF.Tanh, bias_t=b_dfc, name="gamma2")

            w_vg1 = wtile("v_gamma_w", half=1)
            part2 = ap.tile([P, KC], f32, name="part2")
            e2 = gemm([(w_vg1, gamma2)], AF.Exp, accum_t=part2, pre=z2a,
                      name="e2")
            z2 = allreduce6(part2, "z2")

            # --- AllReduce-2 bubble fillers (diff branch tail) --------------
            w_dout1 = wtile("diff_out_w", half=1)
            dout = gemm([(w_dout1, dth)], AF.Tanh, bias_t=bias2, name="dout")
            w_dfus = wtile("diff_fus_w")
            b_dfus = btile("diff_fus_b")
            dfus = gemm([(w_dfus, dout)], AF.Tanh, bias_t=b_dfus, name="dfus")

            s2 = ap.tile([P, KC], f32, name="s2")
            nc.vector.reciprocal(s2[:], z2[:])
            ag = ap.tile([P, KC, RV], bf16, name="ag")
            for mc in range(KC):
                nc.vector.scalar_tensor_tensor(
                    ag[:, mc, :], e2[:, mc, :], s2[:, mc:mc + 1],
                    vanT[:, mc, :], op0=ALU.mult, op1=ALU.mult)

            w_vo1 = wtile("van_out_w", half=1)
            vout = gemm([(w_vo1, ag)], AF.Tanh, pre=voa, name="vout")
            w_vfus = wtile("van_fus_w")
            b_vfus = btile("van_fus_b")
            vfus = gemm([(w_vfus, vout)], AF.Tanh, bias_t=b_vfus, name="vfus")

            # gate (M=1 GEMM over both fusion tensors)
            def vec_unit(wname, act_pairs, name):
                wt = wsp.tile([P, 2 * KC, 1], bf16, name=f"ws_{name}", tag="ws")
                nc.sync.dma_start(wt[:], wd[wname].rearrange(
                    "(c p) o -> p c o", p=P))
                ps = pp.tile([1, RV], f32, name=f"{name}ps", tag="sps", bufs=3)
                i = 0
                for at, base in act_pairs:
                    for kc in range(KC):
                        nc.tensor.matmul(ps[:], wt[:, base + kc, :],
                                         at[:, kc, :],
                                         start=(i == 0), stop=(i == 2 * KC - 1))
                        i += 1
                out = ap.tile([1, RV], f32, name=f"v_{name}")
                nc.scalar.activation(out[:], ps[:], AF.Sigmoid)
                return out

            g = vec_unit("gate_w", [(dfus, 0), (vfus, KC)], "gate")
            gbc = pp.tile([P, RV], f32, name="gbc", tag="sps", bufs=3)
            nc.tensor.matmul(gbc[:], ones128[:], g[:], start=True, stop=True)

            fus = ap.tile([P, KC, RV], bf16, name="fus")
            for mc in range(KC):
                t1 = lp.tile([P, RV], bf16, name=f"ft1_{mc}", tag="ft1")
                nc.vector.tensor_sub(t1[:], vfus[:, mc, :], dfus[:, mc, :])
                t2 = lp.tile([P, RV], bf16, name=f"ft2_{mc}", tag="ft2")
                nc.vector.tensor_mul(t2[:], t1[:], gbc[:])
                nc.vector.tensor_add(fus[:, mc, :], t2[:], dfus[:, mc, :])

            w_nf = wtile("nf_w")
            b_nf = btile("nf_b")
            tnf = gemm([(w_nf, fus)], AF.Identity, bias_t=b_nf, name="tnf")
            nfv = vec_unit("nf_out_w", [(vanT, 0), (tnf, KC)], "nf")
            nbc = pp.tile([P, RV], f32, name="nbc", tag="sps", bufs=3)
            nc.tensor.matmul(nbc[:], ones128[:], nfv[:], start=True, stop=True)

            w_fin = wtile("final_w")
            b_fin = btile("final_b")
            ft = gemm([(w_fin, fus)], AF.Tanh, bias_t=b_fin, name="ftanh")
            outT = ap.tile([P, KC, RV], f32, name="outT")
            for mc in range(KC):
                nc.vector.tensor_mul(outT[:, mc, :], ft[:, mc, :], nbc[:])
            nc.sync.dma_start(out_d.rearrange("(mc p) n -> p mc n", p=P), outT[:])

    nc.compile()
    return nc


_CACHE = {}


def kernel(**inputs):
    x = np.asarray(inputs["x"], np.float32)
    y = np.asarray(inputs["y"], np.float32)
    has_vvb = bool(np.any(np.asarray(inputs["vv_b"]) != 0))
    if has_vvb not in _CACHE:
        _CACHE[has_vvb] = build(has_vvb)
    nc = _CACHE[has_vvb]

    xt = np.ascontiguousarray(x.reshape(B * S, H).T).astype(nbf16)   # [H, 2048]
    yts = [np.ascontiguousarray(y[b].T).astype(nbf16) for b in range(B)]

    base = {}
    for w in W768 + W1536 + ["gate_w", "nf_out_w"]:
        base[w] = np.asarray(inputs[w], np.float32).astype(nbf16)
    for b in BIAS:
        base[b] = np.ascontiguousarray(np.asarray(inputs[b], np.float32))
    if has_vvb:
        base["vv_b"] = np.ascontiguousarray(np.asarray(inputs["vv_b"], np.float32))

    in_maps = []
    for c in range(8):
        bat = c // 4
        m = dict(base)
        m["xT"] = np.ascontiguousarray(xt[:, c * RV:(c + 1) * RV])
        m["yT"] = yts[bat]
        in_maps.append(m)

    res = run_bass_kernel_spmd(nc, in_maps, core_ids=list(range(8)))
    full = np.concatenate([res.results[c]["outT"] for c in range(8)], axis=1)
    return np.ascontiguousarray(full.T.reshape(B, S, H)).astype(np.float32)


if __name__ == "__main__":
    rng = np.random.default_rng(0)
    ins = {"x": rng.standard_normal((B, S, H)).astype(np.float32),
           "y": rng.standard_normal((B, S, H)).astype(np.float32)}
    for w in W768 + W1536:
        shp = (H, H) if w in W768 else (2 * H, H)
        ins[w] = (rng.standard_normal(shp) * 0.02).astype(np.float32)
    ins["gate_w"] = (rng.standard_normal((2 * H, 1)) * 0.02).astype(np.float32)
    ins["nf_out_w"] = (rng.standard_normal((2 * H, 1)) * 0.02).astype(np.float32)
    for b in BIAS + ["vv_b"]:
        ins[b] = np.zeros(H, np.float32)
    out = kernel(**ins)
    print("out", out.shape, out.dtype, np.abs(out).mean())



# revision 10
# speedup vs baseline: 1.1582x; 1.1582x over previous
"""Coupled-attention module as a distributed Bass/Tile kernel on 8 TRN2 cores.

Math notes (exact algebra, not approximations):
- The differential-attention scores are constant along the softmax axis, so
  softmax yields exactly uniform 1/S weights: diff_vector collapses to the
  per-batch mean of (y @ dv_w + dv_b), broadcast over sequence. dq/dk are dead.
- Sharding: rows of the flattened (B*S, H) activations, 256 per core; cores
  0-3 own batch 0, 4-7 batch 1. Each core redundantly computes full-batch K/V
  (cheaper than any reshard collective at this scale).
- All activations live channel-major [C, rows] on chip, so weights feed the
  PE as natural [K, M] lhsT tiles, and the two sequence-axis softmaxes in the
  gating network reduce along the free dim. Their denominators are summed
  across the 4-core batch group with tiny AllReduces.
- Compute in bf16 with fp32 accumulation (all GEMMs), exp/tanh on ACT.
- Attention softmax normalization is deferred: per head the unnormalized
  PV and the Z row are copied out, then ONE batched reciprocal [12, 256]
  plus a selector-matmul broadcast rescales all heads (the per-head DVE
  reciprocal at [1, 256] costs the same as the whole batch).
- Sigmoids are computed as 0.5*(1+tanh(x/2)) to stay in the exp/tanh ACT
  table set (avoids a ~1.5us ACT table switch).
- no_sync_barrier after each AllReduce input keeps the filler GEMMs from
  being consumed early, so they land inside the collective's window.
"""

import numpy as np
import ml_dtypes

import concourse.bass as bass
import concourse.mybir as mybir
import concourse.tile as tile
from concourse import bacc
from concourse.bass_utils import run_bass_kernel_spmd

B, S, H = 2, 1024, 768
NH, DH = 12, 64
P = 128
RV = 256            # rows per core
KC = H // P         # 6 channel chunks
JC = S // P         # 8 sequence chunks
GROUPS = [[0, 1, 2, 3], [4, 5, 6, 7]]
SCALE = 1.0 / 8.0   # 1/sqrt(DH)

bf16 = mybir.dt.bfloat16
f32 = mybir.dt.float32
AF = mybir.ActivationFunctionType
ALU = mybir.AluOpType
nbf16 = ml_dtypes.bfloat16

W768 = ["vq_w", "vk_w", "vv_w", "dv_w", "WD_w", "van_fc_w", "WV_w", "diff_fc_w",
        "diff_fus_w", "van_fus_w", "nf_w", "final_w"]
W1536 = ["d_theta_w", "v_gamma_w", "diff_out_w", "van_out_w"]
BIAS = ["vq_b", "vk_b", "dv_b", "van_fc_b", "d_theta_b", "diff_fc_b",
        "v_gamma_b", "diff_out_b", "van_out_b", "diff_fus_b", "van_fus_b",
        "nf_b", "final_b"]


def build(has_vvb: bool):
    nc = bacc.Bacc(None, target_bir_lowering=False, debug=False, num_devices=8)

    xT_d = nc.dram_tensor("xT", [H, RV], bf16, kind="ExternalInput")
    yT_d = nc.dram_tensor("yT", [H, S], bf16, kind="ExternalInput")
    wd = {}
    for w in W768:
        wd[w] = nc.dram_tensor(w, [H, H], bf16, kind="ExternalInput")
    for w in W1536:
        wd[w] = nc.dram_tensor(w, [2 * H, H], bf16, kind="ExternalInput")
    wd["gate_w"] = nc.dram_tensor("gate_w", [2 * H, 1], bf16, kind="ExternalInput")
    wd["nf_out_w"] = nc.dram_tensor("nf_out_w", [2 * H, 1], bf16, kind="ExternalInput")
    bd = {}
    for b in BIAS:
        bd[b] = nc.dram_tensor(b, [H], f32, kind="ExternalInput")
    if has_vvb:
        bd["vv_b"] = nc.dram_tensor("vv_b", [H], f32, kind="ExternalInput")
    sel_d = nc.dram_tensor("selM", [NH, KC * P], bf16, kind="ExternalInput")
    out_d = nc.dram_tensor("outT", [H, RV], f32, kind="ExternalOutput")

    with tile.TileContext(nc, num_cores=8) as tc:
        with (
            tc.tile_pool(name="wpool", bufs=6) as wp,
            tc.tile_pool(name="wsmall", bufs=2) as wsp,
            tc.tile_pool(name="acts", bufs=1) as ap,
            tc.tile_pool(name="loop", bufs=2) as lp,
            tc.tile_pool(name="psum", bufs=8, space="PSUM") as pp,
            tc.tile_pool(name="dram", bufs=4, space="DRAM") as dp,
        ):
            # PSUM budget: tag "ps" = 6 x 1-bank [128, 512] f32 slots shared
            # by every projection/score/gating matmul; tag "pv" = 2 x 1-bank
            # [65, 2, 256] accumulators. 6 + 2 = 8 banks exactly.
            def psum(shape, name):
                return pp.tile(shape, f32, name=name, tag="ps", bufs=6)

            def wtile(name, half=None):
                t = wp.tile([P, KC, H], bf16, name=f"w_{name}_{half}", tag="w")
                src = wd[name]
                if half is not None:
                    src = src[half * H:(half + 1) * H, :]
                src = src.rearrange("(kc p) n -> kc p n", p=P)
                for kc in range(KC):
                    nc.sync.dma_start(t[:, kc, :], src[kc])
                return t

            def btile(name):
                t = ap.tile([P, KC], f32, name=f"b_{name}")
                nc.sync.dma_start(t[:], bd[name].rearrange("(c p) -> p c", p=P))
                return t

            # ---------------- Q projection first: minimal-dependency PE work
            b_vq = btile("vq_b")
            xT = ap.tile([P, KC, RV], bf16, name="xT")
            for kc in range(KC):
                nc.sync.dma_start(xT[:, kc, :], xT_d.rearrange(
                    "(kc p) n -> kc p n", p=P)[kc])
            w_vq = wtile("vq_w")
            qT = ap.tile([P, KC, RV], bf16, name="qT")
            for mc in range(KC):
                ps = psum([P, RV], f"qps{mc}")
                for kc in range(KC):
                    nc.tensor.matmul(ps[:], w_vq[:, kc, mc * P:(mc + 1) * P],
                                     xT[:, kc, :],
                                     start=(kc == 0), stop=(kc == KC - 1))
                nc.scalar.activation(qT[:, mc, :], ps[:], AF.Identity,
                                     bias=b_vq[:, mc:mc + 1])

            b_vk = btile("vk_b")
            b_dv = btile("dv_b")
            yT = ap.tile([P, KC, S], bf16, name="yT")
            for kc in range(KC):
                nc.sync.dma_start(yT[:, kc, :], yT_d.rearrange(
                    "(kc p) n -> kc p n", p=P)[kc])

            ones128 = ap.tile([1, P], f32, name="ones128")
            nc.vector.memset(ones128[:], 1.0)

            # selector for broadcasting invZ rows (heads) onto channel chunks
            sel = ap.tile([NH, KC, P], bf16, name="sel")
            nc.sync.dma_start(sel[:], sel_d.rearrange("h (c p) -> h c p", p=P))

            # ---------------- K projection (full batch, channel-major) ------
            w_vk = wtile("vk_w")
            kT = ap.tile([P, KC, S], bf16, name="kT")
            for mc in range(KC):
                for nh in range(2):
                    ps = psum([P, 512], f"kps{mc}_{nh}")
                    for kc in range(KC):
                        nc.tensor.matmul(
                            ps[:], w_vk[:, kc, mc * P:(mc + 1) * P],
                            yT[:, kc, nh * 512:(nh + 1) * 512],
                            start=(kc == 0), stop=(kc == KC - 1))
                    nc.scalar.activation(kT[:, mc, nh * 512:(nh + 1) * 512], ps[:],
                                         AF.Identity, bias=b_vk[:, mc:mc + 1])

            # ---------------- V projection (row-major + ones col) -----------
            w_vv = wtile("vv_w")
            v_aug = ap.tile([P, JC, NH, DH + 1], bf16, name="v_aug")
            nc.vector.memset(v_aug[:, :, :, DH:DH + 1], 1.0)
            for cg in range(2):
                for jc in range(JC):
                    ps = psum([P, 384], f"vps{jc}_{cg}")
                    for kc in range(KC):
                        nc.tensor.matmul(
                            ps[:], yT[:, kc, jc * P:(jc + 1) * P],
                            w_vv[:, kc, cg * 384:(cg + 1) * 384],
                            start=(kc == 0), stop=(kc == KC - 1))
                    nc.vector.tensor_copy(
                        v_aug[:, jc, cg * 6:(cg + 1) * 6, 0:DH],
                        ps[:].rearrange("p (h d) -> p h d", d=DH))

            # ---------------- diff-branch constants (per batch) -------------
            # m = mean_s(y) @ dv_w + dv_b ; theta1 = tanh(m @ WD_w)
            # bias1 = theta1 @ d_theta_w[:H] + d_theta_b
            # bias2 = m @ diff_out_w[:H] + diff_out_b
            yb = ap.tile([P, KC], f32, name="yb")
            ybt = ap.tile([P, KC], bf16, name="ybt")
            for kc in range(KC):
                nc.vector.tensor_reduce(yb[:, kc:kc + 1], yT[:, kc, :],
                                        axis=mybir.AxisListType.X, op=ALU.add)
            nc.vector.tensor_scalar_mul(ybt[:], yb[:], 1.0 / S)

            def vec_chain(w_t, rhs_t, func, bias_t, out_dt, name):
                out = ap.tile([P, KC], out_dt, name=name)
                for mc in range(KC):
                    ps = psum([P, 1], f"{name}ps{mc}")
                    for kc in range(KC):
                        nc.tensor.matmul(ps[:], w_t[:, kc, mc * P:(mc + 1) * P],
                                         rhs_t[:, kc:kc + 1],
                                         start=(kc == 0), stop=(kc == KC - 1))
                    nc.scalar.activation(out[:, mc:mc + 1], ps[:], func,
                                         bias=(bias_t[:, mc:mc + 1]
                                               if bias_t is not None else 0.0))
                return out

            w_dv = wtile("dv_w")
            m32 = vec_chain(w_dv, ybt, AF.Identity, b_dv, f32, "m32")
            mbf = ap.tile([P, KC], bf16, name="mbf")
            nc.vector.tensor_copy(mbf[:], m32[:])
            w_WD = wtile("WD_w")
            th1 = vec_chain(w_WD, mbf, AF.Tanh, None, bf16, "th1")
            w_dth0 = wtile("d_theta_w", half=0)
            b_dth = btile("d_theta_b")
            bias1 = vec_chain(w_dth0, th1, AF.Identity, b_dth, f32, "bias1")
            w_dout0 = wtile("diff_out_w", half=0)
            b_dout = btile("diff_out_b")
            bias2 = vec_chain(w_dout0, mbf, AF.Identity, b_dout, f32, "bias2")

            # ---------------- attention (12 heads, 256 own queries) ---------
            # Unnormalized PV + Z rows per head; normalization batched after.
            if has_vvb:
                b_vv = btile("vv_b")
            van_un = ap.tile([P, KC, RV], bf16, name="van_un")
            zcat = ap.tile([1, NH, RV], f32, name="zcat")
            vanT = ap.tile([P, KC, RV], bf16, name="vanT")

            for hp in range(NH // 2):
                h0, h1 = 2 * hp, 2 * hp + 1
                hc = hp
                e0 = lp.tile([P, JC, RV], bf16, name=f"expT{h0}", tag="expT",
                             bufs=3)
                e1_ = lp.tile([P, JC, RV], bf16, name=f"expT{h1}", tag="expT",
                              bufs=3)
                for half in range(4):
                    sc0 = psum([P, 2, RV], f"sc{h0}_{half}")
                    sc1 = psum([P, 2, RV], f"sc{h1}_{half}")
                    for jj in range(2):
                        jc = half * 2 + jj
                        # h0 on PE row groups 0-1, h1 on 2-3: adjacent issue
                        # lets the two K=64 matmuls overlap in the array.
                        nc.tensor.matmul(sc0[:, jj, :],
                                         kT[0:DH, hc, jc * P:(jc + 1) * P],
                                         qT[0:DH, hc, :],
                                         start=True, stop=True)
                        nc.tensor.matmul(sc1[:, jj, :],
                                         kT[DH:P, hc, jc * P:(jc + 1) * P],
                                         qT[DH:P, hc, :],
                                         start=True, stop=True)
                    nc.scalar.activation(e0[:, half * 2:half * 2 + 2, :],
                                         sc0[:], AF.Exp, scale=SCALE)
                    nc.scalar.activation(e1_[:, half * 2:half * 2 + 2, :],
                                         sc1[:], AF.Exp, scale=SCALE)
                pv0 = pp.tile([DH + 1, RV], f32, name=f"pv{h0}", tag="pv",
                              bufs=2)
                pv1 = pp.tile([DH + 1, RV], f32, name=f"pv{h1}", tag="pv",
                              bufs=2)
                for jc in range(JC):
                    nc.tensor.matmul(pv0[:], v_aug[:, jc, h0, :],
                                     e0[:, jc, :],
                                     start=(jc == 0), stop=(jc == JC - 1))
                    nc.tensor.matmul(pv1[:], v_aug[:, jc, h1, :],
                                     e1_[:, jc, :],
                                     start=(jc == 0), stop=(jc == JC - 1))
                nc.vector.tensor_copy(van_un[0:DH, hc, :], pv0[0:DH, :])
                nc.vector.tensor_copy(van_un[DH:P, hc, :], pv1[0:DH, :])
                nc.vector.tensor_copy(zcat[0:1, h0, :], pv0[DH:DH + 1, :])
                nc.vector.tensor_copy(zcat[0:1, h1, :], pv1[DH:DH + 1, :])

            # batched normalization: one reciprocal for all 12 heads.
            # Engine partition bases must be 32-aligned, so the Z rows are
            # staged along partition 0's free dim and re-laid-out with a
            # local SBUF->SBUF DMA before the reciprocal.
            zall = ap.tile([NH, RV], f32, name="zall")
            nc.sync.dma_start(zall[:], zcat[0:1, :, :])
            invZ = ap.tile([NH, RV], f32, name="invZ")
            nc.vector.reciprocal(invZ[:], zall[:])
            invZb = ap.tile([NH, RV], bf16, name="invZb")
            nc.vector.tensor_copy(invZb[:], invZ[:])
            for hc in range(KC):
                bcp = psum([P, RV], f"bc{hc}")
                nc.tensor.matmul(bcp[:], sel[:, hc, :], invZb[:],
                                 start=True, stop=True)
                bcs = lp.tile([P, RV], bf16, name=f"bcs{hc}", tag="bcs", bufs=2)
                nc.vector.tensor_copy(bcs[:], bcp[:])
                if has_vvb:
                    t0 = lp.tile([P, RV], bf16, name=f"vt{hc}", tag="vt")
                    nc.vector.tensor_mul(t0[:], van_un[:, hc, :], bcs[:])
                    nc.vector.tensor_scalar_add(vanT[:, hc, :], t0[:],
                                                b_vv[:, hc:hc + 1])
                else:
                    nc.vector.tensor_mul(vanT[:, hc, :], van_un[:, hc, :],
                                         bcs[:])

            # ---------------- gating network ---------------------------------
            def gemm(pairs, func, bias_t=None, accum_t=None, name="g",
                     out_dt=bf16, pre=None):
                out = ap.tile([P, KC, RV], out_dt, name=name)
                nmm = len(pairs) * KC
                for mc in range(KC):
                    ps = psum([P, RV], f"{name}ps{mc}")
                    i = 0
                    for wt, at in pairs:
                        for kc in range(KC):
                            nc.tensor.matmul(ps[:],
                                             wt[:, kc, mc * P:(mc + 1) * P],
                                             at[:, kc, :],
                                             start=(i == 0), stop=(i == nmm - 1))
                            i += 1
                    src = ps
                    if pre is not None:
                        tmp = lp.tile([P, RV], f32, name=f"{name}pre{mc}",
                                      tag="pretmp")
                        nc.vector.tensor_add(tmp[:], ps[:], pre[:, mc, :])
                        src = tmp
                    nc.scalar.activation(
                        out[:, mc, :], src[:], func,
                        bias=(bias_t[:, mc:mc + 1] if bias_t is not None else 0.0),
                        accum_out=(accum_t[:, mc:mc + 1]
                                   if accum_t is not None else None))
                return out

            def allreduce6(part, name):
                ci = dp.tile([P, KC], f32, name=f"ci_{name}")
                co = dp.tile([P, KC], f32, name=f"co_{name}")
                nc.sync.dma_start(ci[:], part[:])
                tc.no_sync_barrier()
                nc.gpsimd.collective_compute(
                    "AllReduce", ALU.add, replica_groups=GROUPS,
                    ins=[ci[:]], outs=[co[:]])
                return co

            # weights for the AR1 window fillers load ahead of time
            w_vfc = wtile("van_fc_w")
            b_vfc = btile("van_fc_b")
            w_dth1 = wtile("d_theta_w", half=1)
            w_WV = wtile("WV_w")
            w_vg0 = wtile("v_gamma_w", half=0)
            w_vo0 = wtile("van_out_w", half=0)

            theta2 = gemm([(w_vfc, vanT)], AF.Tanh, bias_t=b_vfc, name="theta2")
            part1 = ap.tile([P, KC], f32, name="part1")
            e1 = gemm([(w_dth1, theta2)], AF.Exp, bias_t=bias1, accum_t=part1,
                      name="e1")
            co1 = allreduce6(part1, "z1")

            # --- AllReduce-1 bubble fillers (independent of z1) -------------
            gamma1 = gemm([(w_WV, vanT)], AF.Tanh, name="gamma1")
            b_vg = btile("v_gamma_b")
            z2a = gemm([(w_vg0, gamma1)], AF.Identity, bias_t=b_vg, name="z2a",
                       out_dt=f32)
            b_vo = btile("van_out_b")
            voa = gemm([(w_vo0, vanT)], AF.Identity, bias_t=b_vo, name="voa",
                       out_dt=f32)
            w_dfc = wtile("diff_fc_w")
            b_dfc = btile("diff_fc_b")
            w_vg1 = wtile("v_gamma_w", half=1)
            w_dout1 = wtile("diff_out_w", half=1)
            w_dfus = wtile("diff_fus_w")

            z1 = ap.tile([P, KC], f32, name="z1")
            nc.sync.dma_start(z1[:], co1[:])
            s1 = ap.tile([P, KC], f32, name="s1")
            nc.vector.reciprocal(s1[:], z1[:])
            nc.vector.tensor_mul(s1[:], s1[:], m32[:])
            dth = ap.tile([P, KC, RV], bf16, name="dth")
            for mc in range(KC):
                nc.vector.tensor_scalar_mul(dth[:, mc, :], e1[:, mc, :],
                                            s1[:, mc:mc + 1])

            gamma2 = gemm([(w_dfc, dth)], AF.Tanh, bias_t=b_dfc, name="gamma2")

            part2 = ap.tile([P, KC], f32, name="part2")
            e2 = gemm([(w_vg1, gamma2)], AF.Exp, accum_t=part2, pre=z2a,
                      name="e2")
            co2 = allreduce6(part2, "z2")

            # --- AllReduce-2 bubble fillers (diff branch tail) --------------
            b_dfus = btile("diff_fus_b")
            dout = gemm([(w_dout1, dth)], AF.Tanh, bias_t=bias2, name="dout")
            dfus = gemm([(w_dfus, dout)], AF.Tanh, bias_t=b_dfus, name="dfus")
            w_vo1 = wtile("van_out_w", half=1)
            w_vfus = wtile("van_fus_w")
            w_nf = wtile("nf_w")
            w_fin = wtile("final_w")

            z2 = ap.tile([P, KC], f32, name="z2")
            nc.sync.dma_start(z2[:], co2[:])
            s2 = ap.tile([P, KC], f32, name="s2")
            nc.vector.reciprocal(s2[:], z2[:])
            ag = ap.tile([P, KC, RV], bf16, name="ag")
            for mc in range(KC):
                nc.vector.scalar_tensor_tensor(
                    ag[:, mc, :], e2[:, mc, :], s2[:, mc:mc + 1],
                    vanT[:, mc, :], op0=ALU.mult, op1=ALU.mult)

            vout = gemm([(w_vo1, ag)], AF.Tanh, pre=voa, name="vout")
            b_vfus = btile("van_fus_b")
            vfus = gemm([(w_vfus, vout)], AF.Tanh, bias_t=b_vfus, name="vfus")

            # gate: sigmoid(u) = 0.5*(1+tanh(u/2)); blend uses (1+tanh) bcast
            def vec_unit(wname, act_pairs, name):
                wt = wsp.tile([P, 2 * KC, 1], bf16, name=f"ws_{name}", tag="ws")
                nc.sync.dma_start(wt[:], wd[wname].rearrange(
                    "(c p) o -> p c o", p=P))
                ps = psum([1, RV], f"{name}ps")
                i = 0
                for at, base in act_pairs:
                    for kc in range(KC):
                        nc.tensor.matmul(ps[:], wt[:, base + kc, :],
                                         at[:, kc, :],
                                         start=(i == 0), stop=(i == 2 * KC - 1))
                        i += 1
                # t = tanh(u/2); tp1 = 1 + t  (so 0.5*tp1 = sigmoid(u))
                t = ap.tile([1, RV], f32, name=f"v_{name}")
                nc.scalar.activation(t[:], ps[:], AF.Tanh, scale=0.5)
                tp1 = ap.tile([1, RV], f32, name=f"vp_{name}")
                nc.vector.tensor_scalar_add(tp1[:], t[:], 1.0)
                return tp1

            gtp = vec_unit("gate_w", [(dfus, 0), (vfus, KC)], "gate")
            gbp = psum([P, RV], "gbc")
            nc.tensor.matmul(gbp[:], ones128[:], gtp[:], start=True, stop=True)
            gbs = ap.tile([P, RV], bf16, name="gbs")
            nc.vector.tensor_copy(gbs[:], gbp[:])

            # fus = dfus + 0.5*(1+t)*(vfus-dfus)
            fus = ap.tile([P, KC, RV], bf16, name="fus")
            for mc in range(KC):
                t1 = lp.tile([P, RV], bf16, name=f"ft1_{mc}", tag="ft1")
                nc.vector.tensor_sub(t1[:], vfus[:, mc, :], dfus[:, mc, :])
                t2 = lp.tile([P, RV], bf16, name=f"ft2_{mc}", tag="ft2")
                nc.vector.tensor_mul(t2[:], t1[:], gbs[:])
                nc.vector.scalar_tensor_tensor(
                    fus[:, mc, :], t2[:], 0.5, dfus[:, mc, :],
                    op0=ALU.mult, op1=ALU.add)

            b_nf = btile("nf_b")
            tnf = gemm([(w_nf, fus)], AF.Identity, bias_t=b_nf, name="tnf")
            ntp = vec_unit("nf_out_w", [(vanT, 0), (tnf, KC)], "nf")
            nbp = psum([P, RV], "nbc")
            nc.tensor.matmul(nbp[:], ones128[:], ntp[:], start=True, stop=True)
            nbs = ap.tile([P, RV], bf16, name="nbs")
            nc.vector.tensor_copy(nbs[:], nbp[:])

            b_fin = btile("final_b")
            ft = gemm([(w_fin, fus)], AF.Tanh, bias_t=b_fin, name="ftanh")
            outT = ap.tile([P, KC, RV], f32, name="outT")
            for mc in range(KC):
                # out = sigmoid(nf)*tanh(final) = 0.5*(1+t_nf)*ft
                nc.vector.scalar_tensor_tensor(
                    outT[:, mc, :], ft[:, mc, :], 0.5, nbs[:],
                    op0=ALU.mult, op1=ALU.mult)
            nc.sync.dma_start(out_d.rearrange("(mc p) n -> p mc n", p=P), outT[:])

    nc.compile()
    return nc


_CACHE = {}


def _sel_matrix():
    # sel[h, hc*128 + p] = 1 where channel chunk hc's partition p belongs to
    # head h (p < 64 -> head 2*hc, else head 2*hc+1)
    m = np.zeros((NH, KC * P), np.float32)
    for hc in range(KC):
        m[2 * hc, hc * P:hc * P + DH] = 1.0
        m[2 * hc + 1, hc * P + DH:(hc + 1) * P] = 1.0
    return np.ascontiguousarray(m.astype(nbf16))


def kernel(**inputs):
    x = np.asarray(inputs["x"], np.float32)
    y = np.asarray(inputs["y"], np.float32)
    has_vvb = bool(np.any(np.asarray(inputs["vv_b"]) != 0))
    if has_vvb not in _CACHE:
        _CACHE[has_vvb] = build(has_vvb)
    nc = _CACHE[has_vvb]

    xt = np.ascontiguousarray(x.reshape(B * S, H).T).astype(nbf16)   # [H, 2048]
    yts = [np.ascontiguousarray(y[b].T).astype(nbf16) for b in range(B)]

    base = {}
    for w in W768 + W1536 + ["gate_w", "nf_out_w"]:
        base[w] = np.asarray(inputs[w], np.float32).astype(nbf16)
    for b in BIAS:
        base[b] = np.ascontiguousarray(np.asarray(inputs[b], np.float32))
    if has_vvb:
        base["vv_b"] = np.ascontiguousarray(np.asarray(inputs["vv_b"], np.float32))
    base["selM"] = _sel_matrix()

    in_maps = []
    for c in range(8):
        bat = c // 4
        m = dict(base)
        m["xT"] = np.ascontiguousarray(xt[:, c * RV:(c + 1) * RV])
        m["yT"] = yts[bat]
        in_maps.append(m)

    res = run_bass_kernel_spmd(nc, in_maps, core_ids=list(range(8)))
    full = np.concatenate([res.results[c]["outT"] for c in range(8)], axis=1)
    return np.ascontiguousarray(full.T.reshape(B, S, H)).astype(np.float32)


if __name__ == "__main__":
    rng = np.random.default_rng(0)
    ins = {"x": rng.standard_normal((B, S, H)).astype(np.float32),
           "y": rng.standard_normal((B, S, H)).astype(np.float32)}
    for w in W768 + W1536:
        shp = (H, H) if w in W768 else (2 * H, H)
        ins[w] = (rng.standard_normal(shp) * 0.02).astype(np.float32)
    ins["gate_w"] = (rng.standard_normal((2 * H, 1)) * 0.02).astype(np.float32)
    ins["nf_out_w"] = (rng.standard_normal((2 * H, 1)) * 0.02).astype(np.float32)
    for b in BIAS + ["vv_b"]:
        ins[b] = np.zeros(H, np.float32)
    out = kernel(**ins)
    print("out", out.shape, out.dtype, np.abs(out).mean())


# revision 13
# speedup vs baseline: 1.2932x; 1.1166x over previous
"""Coupled-attention module as a distributed Bass/Tile kernel on 8 TRN2 cores.

Math notes (exact algebra, not approximations):
- The differential-attention scores are constant along the softmax axis, so
  softmax yields exactly uniform 1/S weights: diff_vector collapses to the
  per-batch mean of (y @ dv_w + dv_b), broadcast over sequence. dq/dk are dead.
- Sharding: rows of the flattened (B*S, H) activations, 256 per core; cores
  0-3 own batch 0, 4-7 batch 1. Each core redundantly computes full-batch K/V
  (cheaper than any reshard collective at this scale).
- All activations live channel-major [C, rows] on chip, so weights feed the
  PE as natural [K, M] lhsT tiles, and the two sequence-axis softmaxes in the
  gating network reduce along the free dim. Their denominators are summed
  across the 4-core batch group with tiny AllReduces.
- Compute in bf16 with fp32 accumulation (all GEMMs), exp/tanh on ACT.
- Attention softmax normalization is deferred: per head the unnormalized
  PV and the Z row are copied out, then ONE batched reciprocal [12, 256]
  plus a selector-matmul broadcast rescales all heads (the per-head DVE
  reciprocal at [1, 256] costs the same as the whole batch).
- Sigmoids are computed as 0.5*(1+tanh(x/2)) to stay in the exp/tanh ACT
  table set (avoids a ~1.5us ACT table switch).
- no_sync_barrier after each AllReduce input keeps the filler GEMMs from
  being consumed early, so they land inside the collective's window.
"""

import numpy as np
import ml_dtypes

import concourse.bass as bass
import concourse.mybir as mybir
import concourse.tile as tile
from concourse import bacc
from concourse.bass_utils import run_bass_kernel_spmd

B, S, H = 2, 1024, 768
NH, DH = 12, 64
P = 128
RV = 256            # rows per core
KC = H // P         # 6 channel chunks
JC = S // P         # 8 sequence chunks
GROUPS = [[0, 1, 2, 3], [4, 5, 6, 7]]
SCALE = 1.0 / 8.0   # 1/sqrt(DH)

bf16 = mybir.dt.bfloat16
f32 = mybir.dt.float32
AF = mybir.ActivationFunctionType
ALU = mybir.AluOpType
nbf16 = ml_dtypes.bfloat16

W768 = ["vq_w", "vk_w", "vv_w", "dv_w", "WD_w", "van_fc_w", "WV_w", "diff_fc_w",
        "diff_fus_w", "van_fus_w", "nf_w", "final_w"]
W1536 = ["d_theta_w", "v_gamma_w", "diff_out_w", "van_out_w"]
BIAS = ["vq_b", "vk_b", "dv_b", "van_fc_b", "d_theta_b", "diff_fc_b",
        "v_gamma_b", "diff_out_b", "van_out_b", "diff_fus_b", "van_fus_b",
        "nf_b", "final_b"]


def build(has_vvb: bool):
    nc = bacc.Bacc(None, target_bir_lowering=False, debug=False, num_devices=8)

    xT_d = nc.dram_tensor("xT", [H, RV], bf16, kind="ExternalInput")
    yT_d = nc.dram_tensor("yT", [H, S], bf16, kind="ExternalInput")
    wd = {}
    for w in W768:
        wd[w] = nc.dram_tensor(w, [H, H], bf16, kind="ExternalInput")
    for w in W1536:
        wd[w] = nc.dram_tensor(w, [2 * H, H], bf16, kind="ExternalInput")
    wd["gate_w"] = nc.dram_tensor("gate_w", [2 * H, 1], bf16, kind="ExternalInput")
    wd["nf_out_w"] = nc.dram_tensor("nf_out_w", [2 * H, 1], bf16, kind="ExternalInput")
    bd = {}
    for b in BIAS:
        bd[b] = nc.dram_tensor(b, [H], f32, kind="ExternalInput")
    if has_vvb:
        bd["vv_b"] = nc.dram_tensor("vv_b", [H], f32, kind="ExternalInput")
    sel_d = nc.dram_tensor("selM", [NH, KC * P], bf16, kind="ExternalInput")
    out_d = nc.dram_tensor("outT", [H, RV], f32, kind="ExternalOutput")

    with tile.TileContext(nc, num_cores=8) as tc:
        with (
            tc.tile_pool(name="wpool", bufs=6) as wp,
            tc.tile_pool(name="wsmall", bufs=2) as wsp,
            tc.tile_pool(name="acts", bufs=1) as ap,
            tc.tile_pool(name="loop", bufs=2) as lp,
            tc.tile_pool(name="psum", bufs=8, space="PSUM") as pp,
            tc.tile_pool(name="dram", bufs=4, space="DRAM") as dp,
        ):
            # PSUM budget: tag "ps" = 6 x 1-bank [128, 512] f32 slots shared
            # by every projection/score/gating matmul; tag "pv" = 2 x 1-bank
            # [65, 2, 256] accumulators. 6 + 2 = 8 banks exactly.
            def psum(shape, name):
                return pp.tile(shape, f32, name=name, tag="ps", bufs=6)

            def wtile(name, half=None):
                t = wp.tile([P, KC, H], bf16, name=f"w_{name}_{half}", tag="w")
                src = wd[name]
                if half is not None:
                    src = src[half * H:(half + 1) * H, :]
                src = src.rearrange("(kc p) n -> kc p n", p=P)
                for kc in range(KC):
                    nc.sync.dma_start(t[:, kc, :], src[kc])
                return t

            def btile(name):
                t = ap.tile([P, KC], f32, name=f"b_{name}")
                nc.sync.dma_start(t[:], bd[name].rearrange("(c p) -> p c", p=P))
                return t

            # ---------------- Q projection first: minimal-dependency PE work
            b_vq = btile("vq_b")
            xT = ap.tile([P, KC, RV], bf16, name="xT")
            for kc in range(KC):
                nc.sync.dma_start(xT[:, kc, :], xT_d.rearrange(
                    "(kc p) n -> kc p n", p=P)[kc])
            w_vq = wtile("vq_w")
            qT = ap.tile([P, KC, RV], bf16, name="qT")
            for mc in range(KC):
                ps = psum([P, RV], f"qps{mc}")
                for kc in range(KC):
                    nc.tensor.matmul(ps[:], w_vq[:, kc, mc * P:(mc + 1) * P],
                                     xT[:, kc, :],
                                     start=(kc == 0), stop=(kc == KC - 1))
                nc.scalar.activation(qT[:, mc, :], ps[:], AF.Identity,
                                     bias=b_vq[:, mc:mc + 1])

            b_vk = btile("vk_b")
            b_dv = btile("dv_b")
            yT = ap.tile([P, KC, S], bf16, name="yT")
            for kc in range(KC):
                nc.sync.dma_start(yT[:, kc, :], yT_d.rearrange(
                    "(kc p) n -> kc p n", p=P)[kc])

            ones128 = ap.tile([1, P], f32, name="ones128")
            nc.vector.memset(ones128[:], 1.0)

            # selectors for broadcasting invZ rows (heads) onto channel chunks
            selA = ap.tile([8, 4, P], bf16, name="selA")
            nc.sync.dma_start(selA[:], sel_d[0:8, 0:4 * P].rearrange(
                "h (c p) -> h c p", p=P))
            selB = ap.tile([4, 2, P], bf16, name="selB")
            nc.sync.dma_start(selB[:], sel_d[8:NH, 4 * P:KC * P].rearrange(
                "h (c p) -> h c p", p=P))

            # ---------------- K / V projections (emitted interleaved with
            # the attention pairs below so PE stays dense through the
            # ACT-bound exp phases; HAM stays warm) -----------------------
            w_vk = wtile("vk_w")
            kT = ap.tile([P, KC, S], bf16, name="kT")

            def kproj(mc):
                for nh in range(2):
                    ps = psum([P, 512], f"kps{mc}_{nh}")
                    for kc in range(KC):
                        nc.tensor.matmul(
                            ps[:], w_vk[:, kc, mc * P:(mc + 1) * P],
                            yT[:, kc, nh * 512:(nh + 1) * 512],
                            start=(kc == 0), stop=(kc == KC - 1))
                    nc.scalar.activation(kT[:, mc, nh * 512:(nh + 1) * 512], ps[:],
                                         AF.Identity, bias=b_vk[:, mc:mc + 1])

            w_vv = wtile("vv_w")
            v_aug = ap.tile([P, JC, NH, DH + 1], bf16, name="v_aug")
            nc.vector.memset(v_aug[:, :, :, DH:DH + 1], 1.0)

            def vproj(cg):
                for jc in range(JC):
                    ps = psum([P, 384], f"vps{jc}_{cg}")
                    for kc in range(KC):
                        nc.tensor.matmul(
                            ps[:], yT[:, kc, jc * P:(jc + 1) * P],
                            w_vv[:, kc, cg * 384:(cg + 1) * 384],
                            start=(kc == 0), stop=(kc == KC - 1))
                    nc.vector.tensor_copy(
                        v_aug[:, jc, cg * 6:(cg + 1) * 6, 0:DH],
                        ps[:].rearrange("p (h d) -> p h d", d=DH))

            for mc in range(3):
                kproj(mc)

            # ---------------- diff-branch constants (per batch) -------------
            # m = mean_s(y) @ dv_w + dv_b ; theta1 = tanh(m @ WD_w)
            # bias1 = theta1 @ d_theta_w[:H] + d_theta_b
            # bias2 = m @ diff_out_w[:H] + diff_out_b
            yb = ap.tile([P, KC], f32, name="yb")
            ybt = ap.tile([P, KC], bf16, name="ybt")
            for kc in range(KC):
                nc.vector.tensor_reduce(yb[:, kc:kc + 1], yT[:, kc, :],
                                        axis=mybir.AxisListType.X, op=ALU.add)
            nc.vector.tensor_scalar_mul(ybt[:], yb[:], 1.0 / S)

            def vec_chain(w_t, rhs_t, func, bias_t, out_dt, name):
                out = ap.tile([P, KC], out_dt, name=name)
                for mc in range(KC):
                    ps = psum([P, 1], f"{name}ps{mc}")
                    for kc in range(KC):
                        nc.tensor.matmul(ps[:], w_t[:, kc, mc * P:(mc + 1) * P],
                                         rhs_t[:, kc:kc + 1],
                                         start=(kc == 0), stop=(kc == KC - 1))
                    nc.scalar.activation(out[:, mc:mc + 1], ps[:], func,
                                         bias=(bias_t[:, mc:mc + 1]
                                               if bias_t is not None else 0.0))
                return out

            w_dv = wtile("dv_w")
            m32 = vec_chain(w_dv, ybt, AF.Identity, b_dv, f32, "m32")
            mbf = ap.tile([P, KC], bf16, name="mbf")
            nc.vector.tensor_copy(mbf[:], m32[:])
            w_WD = wtile("WD_w")
            th1 = vec_chain(w_WD, mbf, AF.Tanh, None, bf16, "th1")
            w_dth0 = wtile("d_theta_w", half=0)
            b_dth = btile("d_theta_b")
            bias1 = vec_chain(w_dth0, th1, AF.Identity, b_dth, f32, "bias1")
            w_dout0 = wtile("diff_out_w", half=0)
            b_dout = btile("diff_out_b")
            bias2 = vec_chain(w_dout0, mbf, AF.Identity, b_dout, f32, "bias2")

            # ---------------- attention (12 heads, 256 own queries) ---------
            # Unnormalized PV + Z rows per head; normalization batched in two
            # head groups (0-7, 8-11) so vanT chunks 0-3 free up early.
            if has_vvb:
                b_vv = btile("vv_b")
            van_un = ap.tile([P, KC, RV], bf16, name="van_un")
            zcatA = ap.tile([1, 8, RV], f32, name="zcatA")
            zcatB = ap.tile([1, 4, RV], f32, name="zcatB")
            vanT = ap.tile([P, KC, RV], bf16, name="vanT")

            def pair(hp):
                h0, h1 = 2 * hp, 2 * hp + 1
                hc = hp
                zc, zi = (zcatA, 2 * hp) if hp < 4 else (zcatB, 2 * (hp - 4))
                e0 = lp.tile([P, JC, RV], bf16, name=f"expT{h0}", tag="expT",
                             bufs=3)
                e1_ = lp.tile([P, JC, RV], bf16, name=f"expT{h1}", tag="expT",
                              bufs=3)
                for half in range(4):
                    sc0 = psum([P, 2, RV], f"sc{h0}_{half}")
                    sc1 = psum([P, 2, RV], f"sc{h1}_{half}")
                    for jj in range(2):
                        jc = half * 2 + jj
                        # h0 on PE row groups 0-1, h1 on 2-3: adjacent issue
                        # lets the two K=64 matmuls overlap in the array.
                        nc.tensor.matmul(sc0[:, jj, :],
                                         kT[0:DH, hc, jc * P:(jc + 1) * P],
                                         qT[0:DH, hc, :],
                                         start=True, stop=True)
                        nc.tensor.matmul(sc1[:, jj, :],
                                         kT[DH:P, hc, jc * P:(jc + 1) * P],
                                         qT[DH:P, hc, :],
                                         start=True, stop=True)
                    nc.scalar.activation(e0[:, half * 2:half * 2 + 2, :],
                                         sc0[:], AF.Exp, scale=SCALE)
                    nc.scalar.activation(e1_[:, half * 2:half * 2 + 2, :],
                                         sc1[:], AF.Exp, scale=SCALE)
                pv0 = pp.tile([DH + 1, RV], f32, name=f"pv{h0}", tag="pv",
                              bufs=2)
                pv1 = pp.tile([DH + 1, RV], f32, name=f"pv{h1}", tag="pv",
                              bufs=2)
                for jc in range(JC):
                    nc.tensor.matmul(pv0[:], v_aug[:, jc, h0, :],
                                     e0[:, jc, :],
                                     start=(jc == 0), stop=(jc == JC - 1))
                    nc.tensor.matmul(pv1[:], v_aug[:, jc, h1, :],
                                     e1_[:, jc, :],
                                     start=(jc == 0), stop=(jc == JC - 1))
                nc.vector.tensor_copy(van_un[0:DH, hc, :], pv0[0:DH, :])
                nc.vector.tensor_copy(van_un[DH:P, hc, :], pv1[0:DH, :])
                nc.vector.tensor_copy(zc[0:1, zi, :], pv0[DH:DH + 1, :])
                nc.vector.tensor_copy(zc[0:1, zi + 1, :], pv1[DH:DH + 1, :])

            def normalize(zc, nh_, sel_t, hc_list):
                # Engine partition bases must be 32-aligned, so the Z rows
                # are staged along partition 0's free dim and re-laid-out
                # with a local SBUF->SBUF DMA before the reciprocal.
                zall = ap.tile([nh_, RV], f32, name=f"zall{hc_list[0]}")
                nc.sync.dma_start(zall[:], zc[0:1, :, :])
                invZ = ap.tile([nh_, RV], f32, name=f"invZ{hc_list[0]}")
                nc.vector.reciprocal(invZ[:], zall[:])
                invZb = ap.tile([nh_, RV], bf16, name=f"invZb{hc_list[0]}")
                nc.vector.tensor_copy(invZb[:], invZ[:])
                for i, hc in enumerate(hc_list):
                    bcp = psum([P, RV], f"bc{hc}")
                    nc.tensor.matmul(bcp[:], sel_t[:, i, :], invZb[:],
                                     start=True, stop=True)
                    bcs = lp.tile([P, RV], bf16, name=f"bcs{hc}", tag="bcs",
                                  bufs=2)
                    nc.vector.tensor_copy(bcs[:], bcp[:])
                    if has_vvb:
                        t0 = lp.tile([P, RV], bf16, name=f"vt{hc}", tag="vt")
                        nc.vector.tensor_mul(t0[:], van_un[:, hc, :], bcs[:])
                        nc.vector.tensor_scalar_add(vanT[:, hc, :], t0[:],
                                                    b_vv[:, hc:hc + 1])
                    else:
                        nc.vector.tensor_mul(vanT[:, hc, :], van_un[:, hc, :],
                                             bcs[:])

            vproj(0)
            pair(0)
            pair(1)
            for mc in range(3, KC):
                kproj(mc)
            vproj(1)
            pair(2)
            pair(3)
            normalize(zcatA, 8, selA, [0, 1, 2, 3])
            pair(4)
            pair(5)
            normalize(zcatB, 4, selB, [4, 5])

            # ---------------- gating network ---------------------------------
            def gemm(pairs, func, bias_t=None, accum_t=None, name="g",
                     out_dt=bf16, pre=None):
                out = ap.tile([P, KC, RV], out_dt, name=name)
                nmm = len(pairs) * KC
                for mc in range(KC):
                    ps = psum([P, RV], f"{name}ps{mc}")
                    i = 0
                    for wt, at in pairs:
                        for kc in range(KC):
                            nc.tensor.matmul(ps[:],
                                             wt[:, kc, mc * P:(mc + 1) * P],
                                             at[:, kc, :],
                                             start=(i == 0), stop=(i == nmm - 1))
                            i += 1
                    src = ps
                    if pre is not None:
                        tmp = lp.tile([P, RV], f32, name=f"{name}pre{mc}",
                                      tag="pretmp")
                        nc.vector.tensor_add(tmp[:], ps[:], pre[:, mc, :])
                        src = tmp
                    nc.scalar.activation(
                        out[:, mc, :], src[:], func,
                        bias=(bias_t[:, mc:mc + 1] if bias_t is not None else 0.0),
                        accum_out=(accum_t[:, mc:mc + 1]
                                   if accum_t is not None else None))
                return out

            def allreduce6(part, name):
                ci = dp.tile([P, KC], f32, name=f"ci_{name}")
                co = dp.tile([P, KC], f32, name=f"co_{name}")
                nc.sync.dma_start(ci[:], part[:])
                tc.no_sync_barrier()
                nc.gpsimd.collective_compute(
                    "AllReduce", ALU.add, replica_groups=GROUPS,
                    ins=[ci[:]], outs=[co[:]])
                return co

            # weights for the AR1 window fillers load ahead of time
            w_vfc = wtile("van_fc_w")
            b_vfc = btile("van_fc_b")
            w_dth1 = wtile("d_theta_w", half=1)
            w_WV = wtile("WV_w")
            w_vg0 = wtile("v_gamma_w", half=0)
            w_vo0 = wtile("van_out_w", half=0)

            theta2 = gemm([(w_vfc, vanT)], AF.Tanh, bias_t=b_vfc, name="theta2")
            part1 = ap.tile([P, KC], f32, name="part1")
            e1 = gemm([(w_dth1, theta2)], AF.Exp, bias_t=bias1, accum_t=part1,
                      name="e1")
            co1 = allreduce6(part1, "z1")

            # --- AllReduce-1 bubble fillers (independent of z1) -------------
            gamma1 = gemm([(w_WV, vanT)], AF.Tanh, name="gamma1")
            b_vg = btile("v_gamma_b")
            z2a = gemm([(w_vg0, gamma1)], AF.Identity, bias_t=b_vg, name="z2a",
                       out_dt=f32)
            b_vo = btile("van_out_b")
            voa = gemm([(w_vo0, vanT)], AF.Identity, bias_t=b_vo, name="voa",
                       out_dt=f32)
            w_dfc = wtile("diff_fc_w")
            b_dfc = btile("diff_fc_b")
            w_vg1 = wtile("v_gamma_w", half=1)
            w_dout1 = wtile("diff_out_w", half=1)
            w_dfus = wtile("diff_fus_w")

            z1 = ap.tile([P, KC], f32, name="z1")
            nc.sync.dma_start(z1[:], co1[:])
            s1 = ap.tile([P, KC], f32, name="s1")
            nc.vector.reciprocal(s1[:], z1[:])
            nc.vector.tensor_mul(s1[:], s1[:], m32[:])
            dth = ap.tile([P, KC, RV], bf16, name="dth")
            for mc in range(KC):
                nc.vector.tensor_scalar_mul(dth[:, mc, :], e1[:, mc, :],
                                            s1[:, mc:mc + 1])

            gamma2 = gemm([(w_dfc, dth)], AF.Tanh, bias_t=b_dfc, name="gamma2")

            part2 = ap.tile([P, KC], f32, name="part2")
            e2 = gemm([(w_vg1, gamma2)], AF.Exp, accum_t=part2, pre=z2a,
                      name="e2")
            co2 = allreduce6(part2, "z2")

            # --- AllReduce-2 bubble fillers (diff branch tail) --------------
            b_dfus = btile("diff_fus_b")
            dout = gemm([(w_dout1, dth)], AF.Tanh, bias_t=bias2, name="dout")
            dfus = gemm([(w_dfus, dout)], AF.Tanh, bias_t=b_dfus, name="dfus")
            w_vo1 = wtile("van_out_w", half=1)
            w_vfus = wtile("van_fus_w")
            w_nf = wtile("nf_w")
            w_fin = wtile("final_w")

            z2 = ap.tile([P, KC], f32, name="z2")
            nc.sync.dma_start(z2[:], co2[:])
            s2 = ap.tile([P, KC], f32, name="s2")
            nc.vector.reciprocal(s2[:], z2[:])
            ag = ap.tile([P, KC, RV], bf16, name="ag")
            for mc in range(KC):
                nc.vector.scalar_tensor_tensor(
                    ag[:, mc, :], e2[:, mc, :], s2[:, mc:mc + 1],
                    vanT[:, mc, :], op0=ALU.mult, op1=ALU.mult)

            vout = gemm([(w_vo1, ag)], AF.Tanh, pre=voa, name="vout")
            b_vfus = btile("van_fus_b")
            vfus = gemm([(w_vfus, vout)], AF.Tanh, bias_t=b_vfus, name="vfus")

            # gate: sigmoid(u) = 0.5*(1+tanh(u/2)); blend uses (1+tanh) bcast
            def vec_unit(wname, act_pairs, name):
                wt = wsp.tile([P, 2 * KC, 1], bf16, name=f"ws_{name}", tag="ws")
                nc.sync.dma_start(wt[:], wd[wname].rearrange(
                    "(c p) o -> p c o", p=P))
                ps = psum([1, RV], f"{name}ps")
                i = 0
                for at, base in act_pairs:
                    for kc in range(KC):
                        nc.tensor.matmul(ps[:], wt[:, base + kc, :],
                                         at[:, kc, :],
                                         start=(i == 0), stop=(i == 2 * KC - 1))
                        i += 1
                # t = tanh(u/2); tp1 = 1 + t  (so 0.5*tp1 = sigmoid(u))
                t = ap.tile([1, RV], f32, name=f"v_{name}")
                nc.scalar.activation(t[:], ps[:], AF.Tanh, scale=0.5)
                tp1 = ap.tile([1, RV], f32, name=f"vp_{name}")
                nc.vector.tensor_scalar_add(tp1[:], t[:], 1.0)
                return tp1

            gtp = vec_unit("gate_w", [(dfus, 0), (vfus, KC)], "gate")
            gbp = psum([P, RV], "gbc")
            nc.tensor.matmul(gbp[:], ones128[:], gtp[:], start=True, stop=True)
            gbs = ap.tile([P, RV], bf16, name="gbs")
            nc.vector.tensor_copy(gbs[:], gbp[:])

            # fus = dfus + 0.5*(1+t)*(vfus-dfus)
            fus = ap.tile([P, KC, RV], bf16, name="fus")
            for mc in range(KC):
                t1 = lp.tile([P, RV], bf16, name=f"ft1_{mc}", tag="ft1")
                nc.vector.tensor_sub(t1[:], vfus[:, mc, :], dfus[:, mc, :])
                t2 = lp.tile([P, RV], bf16, name=f"ft2_{mc}", tag="ft2")
                nc.vector.tensor_mul(t2[:], t1[:], gbs[:])
                nc.vector.scalar_tensor_tensor(
                    fus[:, mc, :], t2[:], 0.5, dfus[:, mc, :],
                    op0=ALU.mult, op1=ALU.add)

            b_nf = btile("nf_b")
            tnf = gemm([(w_nf, fus)], AF.Identity, bias_t=b_nf, name="tnf")
            ntp = vec_unit("nf_out_w", [(vanT, 0), (tnf, KC)], "nf")
            nbp = psum([P, RV], "nbc")
            nc.tensor.matmul(nbp[:], ones128[:], ntp[:], start=True, stop=True)
            nbs = ap.tile([P, RV], bf16, name="nbs")
            nc.vector.tensor_copy(nbs[:], nbp[:])

            b_fin = btile("final_b")
            ft = gemm([(w_fin, fus)], AF.Tanh, bias_t=b_fin, name="ftanh")
            outT = ap.tile([P, KC, RV], f32, name="outT")
            for mc in range(KC):
                # out = sigmoid(nf)*tanh(final) = 0.5*(1+t_nf)*ft
                nc.vector.scalar_tensor_tensor(
                    outT[:, mc, :], ft[:, mc, :], 0.5, nbs[:],
                    op0=ALU.mult, op1=ALU.mult)
            nc.sync.dma_start(out_d.rearrange("(mc p) n -> p mc n", p=P), outT[:])

    nc.compile()
    return nc


_CACHE = {}


def _sel_matrix():
    # sel[h, hc*128 + p] = 1 where channel chunk hc's partition p belongs to
    # head h (p < 64 -> head 2*hc, else head 2*hc+1)
    m = np.zeros((NH, KC * P), np.float32)
    for hc in range(KC):
        m[2 * hc, hc * P:hc * P + DH] = 1.0
        m[2 * hc + 1, hc * P + DH:(hc + 1) * P] = 1.0
    return np.ascontiguousarray(m.astype(nbf16))


def kernel(**inputs):
    x = np.asarray(inputs["x"], np.float32)
    y = np.asarray(inputs["y"], np.float32)
    has_vvb = bool(np.any(np.asarray(inputs["vv_b"]) != 0))
    if has_vvb not in _CACHE:
        _CACHE[has_vvb] = build(has_vvb)
    nc = _CACHE[has_vvb]

    xt = np.ascontiguousarray(x.reshape(B * S, H).T).astype(nbf16)   # [H, 2048]
    yts = [np.ascontiguousarray(y[b].T).astype(nbf16) for b in range(B)]

    base = {}
    for w in W768 + W1536 + ["gate_w", "nf_out_w"]:
        base[w] = np.asarray(inputs[w], np.float32).astype(nbf16)
    for b in BIAS:
        base[b] = np.ascontiguousarray(np.asarray(inputs[b], np.float32))
    if has_vvb:
        base["vv_b"] = np.ascontiguousarray(np.asarray(inputs["vv_b"], np.float32))
    base["selM"] = _sel_matrix()

    in_maps = []
    for c in range(8):
        bat = c // 4
        m = dict(base)
        m["xT"] = np.ascontiguousarray(xt[:, c * RV:(c + 1) * RV])
        m["yT"] = yts[bat]
        in_maps.append(m)

    res = run_bass_kernel_spmd(nc, in_maps, core_ids=list(range(8)))
    full = np.concatenate([res.results[c]["outT"] for c in range(8)], axis=1)
    return np.ascontiguousarray(full.T.reshape(B, S, H)).astype(np.float32)


if __name__ == "__main__":
    rng = np.random.default_rng(0)
    ins = {"x": rng.standard_normal((B, S, H)).astype(np.float32),
           "y": rng.standard_normal((B, S, H)).astype(np.float32)}
    for w in W768 + W1536:
        shp = (H, H) if w in W768 else (2 * H, H)
        ins[w] = (rng.standard_normal(shp) * 0.02).astype(np.float32)
    ins["gate_w"] = (rng.standard_normal((2 * H, 1)) * 0.02).astype(np.float32)
    ins["nf_out_w"] = (rng.standard_normal((2 * H, 1)) * 0.02).astype(np.float32)
    for b in BIAS + ["vv_b"]:
        ins[b] = np.zeros(H, np.float32)
    out = kernel(**ins)
    print("out", out.shape, out.dtype, np.abs(out).mean())
